# revision 18
# baseline (speedup 1.0000x reference)
"""Trainium2 Bass kernel for nn_LlamaAttention (T=2048, HID=4096, HQ=32, HKV=8, D=128).

Tensor-parallel over heads across 8 NeuronCores: core c owns q-heads 4c..4c+3 and
kv-head c (GQA group size 4 == heads-per-core, so attention is fully core-local).
Wo is row-sharded; each core computes a partial [T, HID] output (transposed) and
the host sums the 8 partials. No device collectives.

All weight/activation inputs are host-pre-swizzled to partition-major layouts so
every DMA is one contiguous run per partition (descriptor-generation on the Sync
engine is the startup bottleneck otherwise).

Self-contained: hardcodes all shapes; builds the Bass kernel once per process.
"""
import numpy as np

T, HID, HQ, HKV, D = 2048, 4096, 32, 8, 128
NCORES = 8
HPC = HQ // NCORES            # 4 q heads per core
QW = HPC * D                  # 512 q columns per core
MW = QW + 2 * D               # 768 qkv columns per core
KO = HID // 128               # 32 k-tiles
NKP = 4                       # phase A K passes
KPQ = KO // NKP               # 8 k-tiles per pass
AC2 = 512                     # phase A T-chunk width
NAC2 = T // AC2               # 4
CH = 512                      # attention tq chunk width
NCH = T // CH                 # 4
SCALING = float(D) ** -0.5

_CACHE = {}


def _build_nc():
    import concourse.mybir as mybir
    import concourse.tile as tile
    from concourse import bacc
    from contextlib import ExitStack

    F32 = mybir.dt.float32
    F32R = mybir.dt.float32r
    AF = mybir.ActivationFunctionType

    nc = bacc.Bacc("TRN2", target_bir_lowering=False, debug=False,
                   dynamic_dma_scratch_size=2048)

    # pre-swizzled inputs (see prep_in_maps)
    hidp = nc.dram_tensor("hidp", [128, NKP * NAC2 * KPQ * AC2], F32R,
                          kind="ExternalInput")
    wqkvp = nc.dram_tensor("wqkvp", [128, KO * MW], F32R, kind="ExternalInput")
    wop = nc.dram_tensor("wop", [128, HPC * (HID // 128) * 128], F32R,
                         kind="ExternalInput")
    cosT = nc.dram_tensor("cosT", [D, T], F32, kind="ExternalInput")
    sinT = nc.dram_tensor("sinT", [D, T], F32, kind="ExternalInput")
    rmatT = nc.dram_tensor("rmatT", [D, D], F32R, kind="ExternalInput")
    onesd = nc.dram_tensor("onesd", [128, 128], F32R, kind="ExternalInput")
    identd = nc.dram_tensor("identd", [128, 128], F32R, kind="ExternalInput")
    masksp = nc.dram_tensor("masksp", [128, 4 * CH], F32, kind="ExternalInput")
    outT = nc.dram_tensor("outT_p", [HID, T], F32, kind="ExternalOutput")

    with tile.TileContext(nc) as tc, ExitStack() as ctx:
        consts = ctx.enter_context(tc.tile_pool(name="consts", bufs=1))
        ones_sb = consts.tile([128, 128], F32R)
        ident = consts.tile([128, 128], F32R)
        rmatT_sb = consts.tile([128, 128], F32R)

        qkv_pool = ctx.enter_context(tc.tile_pool(name="qkvT", bufs=1))
        # m=0..3 q heads, m=4 kT, m=5 vT
        qk_t = [qkv_pool.tile([128, T], F32R, name=f"qkvT{m}") for m in range(6)]
        v_sb = qkv_pool.tile([128, T // 128, 128], F32R, name="v_sb")  # [ts, blk, D]

        csp = ctx.enter_context(tc.tile_pool(name="cs", bufs=1))
        cos_sb = csp.tile([128, T], F32)
        sin_sb = csp.tile([128, T], F32)

        # HAM warmup: PE clock-gate needs ~3.4us of activity to unthrottle.
        # Run dummy fp32 matmuls on a zeroed scratch tile while the first
        # input DMAs are in flight so real work starts at 2.4GHz.
        with tc.tile_pool(name="warm", bufs=1) as wmp, \
             tc.tile_pool(name="psW", bufs=1, space="PSUM") as psW:
            wz = wmp.tile([128, 256], F32)
            nc.vector.memset(wz[:], 0.0)
            pw = psW.tile([128, 256], F32)
            for _ in range(20):
                nc.tensor.matmul(pw[:], wz[:, :128], wz[:], start=True, stop=True)

        # ---- Phase A: qkvT = wqkv.T @ hidT in NKP K-passes, fp32 partials ----
        with tc.tile_pool(name="w", bufs=2) as wp, \
             tc.tile_pool(name="hid", bufs=3) as hp, \
             tc.tile_pool(name="partial", bufs=1) as pp, \
             tc.tile_pool(name="ropetmp", bufs=2) as tmp_pool, \
             tc.tile_pool(name="psA", bufs=3, space="PSUM") as psA, \
             tc.tile_pool(name="psT", bufs=2, space="PSUM") as psT, \
             tc.tile_pool(name="psR", bufs=2, space="PSUM") as psR:
            part = [pp.tile([128, T], F32, name=f"part{m}") for m in range(6)]

            def load_w(kp, split=False):
                wt = wp.tile([128, KPQ * MW], F32R, tag="w", name=f"w{kp}")
                base = kp * KPQ * MW
                if split:
                    q_ = KPQ * MW // 4
                    for qi in range(4):
                        nc.sync.dma_start(wt[:, qi * q_:(qi + 1) * q_],
                                          wqkvp[:, base + qi * q_:base + (qi + 1) * q_])
                else:
                    nc.sync.dma_start(wt[:], wqkvp[:, base:base + KPQ * MW])
                return wt

            def load_hid(kp, ch, split=False):
                ht = hp.tile([128, KPQ * AC2], F32R, tag="hid", name=f"h{kp}_{ch}")
                base = (kp * NAC2 + ch) * KPQ * AC2
                if split:
                    q_ = KPQ * AC2 // 4
                    for qi in range(4):
                        nc.sync.dma_start(ht[:, qi * q_:(qi + 1) * q_],
                                          hidp[:, base + qi * q_:base + (qi + 1) * q_])
                else:
                    nc.sync.dma_start(ht[:], hidp[:, base:base + KPQ * AC2])
                return ht

            h_cur = load_hid(0, 0, split=True)
            w_tiles = {0: load_w(0, split=True)}
            nc.sync.dma_start(ones_sb[:], onesd[:, :])
            nc.sync.dma_start(ident[:], identd[:, :])
            nc.sync.dma_start(rmatT_sb[:], rmatT[:, :])
            w_tiles[1] = load_w(1)
            for kp in range(NKP):
                w_cur = w_tiles.pop(kp)
                for ch in range(NAC2):
                    asl = slice(ch * AC2, (ch + 1) * AC2)
                    if h_cur is None:
                        h_cur = load_hid(kp, ch)
                    if kp == 2 and ch == 0:
                        nc.sync.dma_start(cos_sb[:], cosT[:, :])
                        nc.sync.dma_start(sin_sb[:], sinT[:, :])
                    if ch == 1 and kp + 2 < NKP:
                        w_tiles[kp + 2] = load_w(kp + 2)
                    for m in range(6):
                        ps = psA.tile([128, AC2], F32, tag="psA")
                        for k in range(KPQ):
                            nc.tensor.matmul(
                                ps[:],
                                w_cur[:, k * MW + m * 128:k * MW + (m + 1) * 128],
                                h_cur[:, k * AC2:(k + 1) * AC2],
                                start=(k == 0), stop=(k == KPQ - 1))
                        if kp == 0:
                            nc.vector.tensor_copy(part[m][:, asl], ps[:])
                        elif kp < NKP - 1:
                            nc.vector.tensor_add(part[m][:, asl], part[m][:, asl],
                                                 ps[:])
                        else:
                            # final pass: single rounding into f32r qkvT
                            nc.vector.tensor_add(qk_t[m][:, asl], part[m][:, asl],
                                                 ps[:])
                            if m < 5:
                                # fused RoPE: x = x*cos + (rmatT.T@x)*sin
                                rp = psR.tile([128, AC2], F32, tag="psR")
                                nc.tensor.matmul(rp[:], rmatT_sb[:], qk_t[m][:, asl],
                                                 start=True, stop=True)
                                tmp = tmp_pool.tile([128, AC2], F32, tag="tmp")
                                nc.vector.tensor_mul(tmp[:], rp[:], sin_sb[:, asl])
                                nc.vector.tensor_mul(qk_t[m][:, asl],
                                                     qk_t[m][:, asl], cos_sb[:, asl])
                                nc.vector.tensor_add(qk_t[m][:, asl],
                                                     qk_t[m][:, asl], tmp[:])
                            else:
                                for jj in range(AC2 // 128):
                                    j = ch * (AC2 // 128) + jj
                                    pst = psT.tile([128, 128], F32R, tag="psT")
                                    nc.tensor.transpose(
                                        pst[:], qk_t[5][:, j * 128:(j + 1) * 128],
                                        ident[:])
                                    nc.vector.tensor_copy(v_sb[:, j, :], pst[:])
                    h_cur = None

        # ---------------- Phase B consts ----------------
        bconsts = ctx.enter_context(tc.tile_pool(name="bconsts", bufs=1))
        mask_sb = bconsts.tile([128, 4 * CH], F32)
        nc.sync.dma_start(mask_sb[:], masksp[:, :])

        wopool = ctx.enter_context(tc.tile_pool(name="wo", bufs=1))
        wo_sb = wopool.tile([128, HPC * (HID // 128) * 128], F32R)
        nc.sync.dma_start(wo_sb[:], wop[:, :])

        probs_pool = ctx.enter_context(tc.tile_pool(name="probs", bufs=5))
        attn_pool = ctx.enter_context(tc.tile_pool(name="attnT", bufs=1))
        attn_sb = [attn_pool.tile([128, T], F32R, name=f"attnT{h}") for h in range(HPC)]
        rden_pool = ctx.enter_context(tc.tile_pool(name="rden", bufs=2))
        out_pool = ctx.enter_context(tc.tile_pool(name="outstage", bufs=4))
        psS = ctx.enter_context(tc.tile_pool(name="psS", bufs=2, space="PSUM"))
        psAcc = ctx.enter_context(tc.tile_pool(name="psAcc", bufs=2, space="PSUM"))
        psD = ctx.enter_context(tc.tile_pool(name="psD", bufs=2, space="PSUM"))
        psO = ctx.enter_context(tc.tile_pool(name="psO", bufs=2, space="PSUM"))

        # ---------------- Phase B+C per tq chunk (C deferred one chunk) ---
        def phase_c(c):
            sl = slice(c * CH, (c + 1) * CH)
            for mo in range(HID // 128):
                po = psO.tile([128, CH], F32, tag="po")
                for kk in range(HPC):
                    woff = (kk * (HID // 128) + mo) * 128
                    nc.tensor.matmul(po[:], wo_sb[:, woff:woff + 128],
                                     attn_sb[kk][:, sl],
                                     start=(kk == 0), stop=(kk == HPC - 1))
                ob = out_pool.tile([128, CH], F32, tag="ob")
                if mo % 2 == 0:
                    nc.scalar.copy(ob[:], po[:])
                else:
                    nc.vector.tensor_copy(ob[:], po[:])
                nc.sync.dma_start(outT[mo * 128:(mo + 1) * 128, sl], ob[:])

        for c in range(NCH):
            sl = slice(c * CH, (c + 1) * CH)
            nts = (CH // 128) * (c + 1)  # causal: ts tiles 0..nts-1
            for h in range(HPC):
                pa = psAcc.tile([128, CH], F32, tag="acc")
                pd = psD.tile([128, CH], F32, tag="den")
                for j in range(nts):
                    i = j - (CH // 128) * c
                    # diagonal tiles: cols < i*128 fully masked; shrink to
                    # [off:512] (off capped at 256 to keep f32r N>=256 fast)
                    off = 0 if i < 1 else min(i * 128, 256)
                    csl = slice(c * CH + off, (c + 1) * CH)
                    ss = psS.tile([128, CH], F32, tag="psS")
                    nc.tensor.matmul(ss[:, off:], qk_t[4][:, j * 128:(j + 1) * 128],
                                     qk_t[h][:, csl], start=True, stop=True)
                    pr = probs_pool.tile([128, CH], F32R, tag="probs")
                    nc.scalar.activation(pr[:, off:], ss[:, off:], AF.Exp,
                                         scale=SCALING)
                    if i >= 0:
                        nc.vector.tensor_mul(pr[:, off:], pr[:, off:],
                                             mask_sb[:, i * CH + off:(i + 1) * CH])
                    nc.tensor.matmul(pd[:, off:], ones_sb[:], pr[:, off:],
                                     start=(j == 0), stop=(j == nts - 1))
                    nc.tensor.matmul(pa[:, off:], v_sb[:, j, :], pr[:, off:],
                                     start=(j == 0), stop=(j == nts - 1))
                rden = rden_pool.tile([128, CH], F32, tag="rden")
                nc.vector.reciprocal_approx_fast(out=rden[:], in_=pd[:])
                nc.vector.tensor_mul(attn_sb[h][:, sl], pa[:], rden[:])
            if c >= 1:
                phase_c(c - 1)
        phase_c(NCH - 1)

    nc.compile()
    return nc


def get_nc():
    if "nc" not in _CACHE:
        _CACHE["nc"] = _build_nc()
    return _CACHE["nc"]


def prep_in_maps(hidden_states, cos, sin, Wq, Wk, Wv, Wo):
    # hidp[p, (((kp*NAC2)+ch)*KPQ + ko)*AC2 + t] = hidT[(kp*KPQ+ko)*128+p, ch*AC2+t]
    hidT = hidden_states.T  # [HID, T]
    hidp = (hidT.reshape(NKP, KPQ, 128, NAC2, AC2)
            .transpose(2, 0, 3, 1, 4).reshape(128, -1))
    hidp = np.ascontiguousarray(hidp)
    cosT = np.ascontiguousarray(cos.T)
    sinT = np.ascontiguousarray(sin.T)
    # signed rotate-half permutation (as lhsT): rot = rmatT.T @ x
    rmatT = np.zeros((D, D), dtype=np.float32)
    half = D // 2
    rmatT[np.arange(half) + half, np.arange(half)] = -1.0  # rot[d<64] = -x[d+64]
    rmatT[np.arange(half), np.arange(half) + half] = 1.0   # rot[d>=64] = x[d-64]
    # masksp[p, i*CH+f] = 1 if i*128+p <= f
    i_idx = np.arange(4)[None, :, None] * 128
    p_idx = np.arange(128)[:, None, None]
    f_idx = np.arange(CH)[None, None, :]
    masksp = ((i_idx + p_idx) <= f_idx).astype(np.float32).reshape(128, -1)

    in_maps = []
    for c in range(NCORES):
        wqkv = np.concatenate([
            Wq[:, c * QW:(c + 1) * QW],
            Wk[:, c * D:(c + 1) * D],
            Wv[:, c * D:(c + 1) * D],
        ], axis=1)  # [HID, MW]
        # wqkvp[p, ko*MW+m] = wqkv[ko*128+p, m]
        wqkvp = np.ascontiguousarray(
            wqkv.reshape(KO, 128, MW).transpose(1, 0, 2).reshape(128, -1))
        # wop[p, (kk*(HID//128)+mo)*128+q] = Wo[c*QW + kk*128+p, mo*128+q]
        wo_c = Wo[c * QW:(c + 1) * QW, :]
        wop_ = np.ascontiguousarray(
            wo_c.reshape(HPC, 128, HID // 128, 128).transpose(1, 0, 2, 3)
            .reshape(128, -1))
        in_maps.append({
            "hidp": hidp,
            "wqkvp": wqkvp,
            "wop": wop_,
            "cosT": cosT,
            "sinT": sinT,
            "rmatT": rmatT,
            "onesd": np.ones((128, 128), dtype=np.float32),
            "identd": np.eye(128, dtype=np.float32),
            "masksp": masksp,
        })
    return in_maps


def postprocess(results):
    acc = results[0]["outT_p"].copy()
    for r in results[1:]:
        acc += r["outT_p"]
    return np.ascontiguousarray(acc.T).astype(np.float32)


def kernel(hidden_states, position_ids, cos, sin, Wq, Wk, Wv, Wo):
    from concourse.bass_utils import run_bass_kernel_spmd
    nc = get_nc()
    in_maps = prep_in_maps(hidden_states, cos, sin, Wq, Wk, Wv, Wo)
    res = run_bass_kernel_spmd(nc, in_maps, core_ids=list(range(NCORES)))
    return postprocess(res.results)


# revision 19
# speedup vs baseline: 1.1487x; 1.1487x over previous
"""Trainium2 Bass kernel for nn_LlamaAttention (T=2048, HID=4096, HQ=32, HKV=8, D=128).

Tensor-parallel over heads across 8 NeuronCores: core c owns q-heads 4c..4c+3 and
kv-head c (GQA group size 4 == heads-per-core, so attention is fully core-local).
Wo is row-sharded; each core computes a partial [T, HID] output (transposed) and
the host sums the 8 partials. No device collectives.

All weight/activation inputs are host-pre-swizzled to partition-major layouts so
every DMA is one contiguous run per partition (descriptor-generation on the Sync
engine is the startup bottleneck otherwise).

Self-contained: hardcodes all shapes; builds the Bass kernel once per process.
"""
import numpy as np

T, HID, HQ, HKV, D = 2048, 4096, 32, 8, 128
NCORES = 8
HPC = HQ // NCORES            # 4 q heads per core
QW = HPC * D                  # 512 q columns per core
MW = QW + 2 * D               # 768 qkv columns per core
KO = HID // 128               # 32 k-tiles
NKP = 4                       # phase A K passes
KPQ = KO // NKP               # 8 k-tiles per pass
AC2 = 512                     # phase A T-chunk width
NAC2 = T // AC2               # 4
CH = 512                      # attention tq chunk width
NCH = T // CH                 # 4
SCALING = float(D) ** -0.5

_CACHE = {}


def _build_nc():
    import concourse.mybir as mybir
    import concourse.tile as tile
    from concourse import bacc
    from contextlib import ExitStack

    F32 = mybir.dt.float32
    F32R = mybir.dt.float32r
    AF = mybir.ActivationFunctionType

    nc = bacc.Bacc("TRN2", target_bir_lowering=False, debug=False,
                   dynamic_dma_scratch_size=2048)

    # pre-swizzled inputs (see prep_in_maps)
    hidp = nc.dram_tensor("hidp", [128, NKP * NAC2 * KPQ * AC2], F32R,
                          kind="ExternalInput")
    wqkvp = nc.dram_tensor("wqkvp", [128, KO * MW], F32R, kind="ExternalInput")
    wop = nc.dram_tensor("wop", [128, HPC * (HID // 128) * 128], F32R,
                         kind="ExternalInput")
    cosT = nc.dram_tensor("cosT", [D, T], F32, kind="ExternalInput")
    sinT = nc.dram_tensor("sinT", [D, T], F32, kind="ExternalInput")
    rmatT = nc.dram_tensor("rmatT", [D, D], F32R, kind="ExternalInput")
    onesd = nc.dram_tensor("onesd", [128, 128], F32R, kind="ExternalInput")
    identd = nc.dram_tensor("identd", [128, 128], F32R, kind="ExternalInput")
    masksp = nc.dram_tensor("masksp", [128, 4 * CH], F32, kind="ExternalInput")
    outT = nc.dram_tensor("outT_p", [HID, T], F32, kind="ExternalOutput")

    with tile.TileContext(nc) as tc, ExitStack() as ctx:
        consts = ctx.enter_context(tc.tile_pool(name="consts", bufs=1))
        ones_sb = consts.tile([128, 128], F32R)
        ident = consts.tile([128, 128], F32R)
        rmatT_sb = consts.tile([128, 128], F32R)

        qkv_pool = ctx.enter_context(tc.tile_pool(name="qkvT", bufs=1))
        # m=0..3 q heads, m=4 kT, m=5 vT
        qk_t = [qkv_pool.tile([128, T], F32R, name=f"qkvT{m}") for m in range(6)]
        v_sb = qkv_pool.tile([128, T // 128, 128], F32R, name="v_sb")  # [ts, blk, D]

        csp = ctx.enter_context(tc.tile_pool(name="cs", bufs=1))
        cos_sb = csp.tile([128, T], F32)
        sin_sb = csp.tile([128, T], F32)

        # ---- Phase A: qkvT = wqkv.T @ hidT in NKP K-passes, fp32 partials ----
        with tc.tile_pool(name="w", bufs=2) as wp, \
             tc.tile_pool(name="hid", bufs=3) as hp, \
             tc.tile_pool(name="partial", bufs=1) as pp, \
             tc.tile_pool(name="ropetmp", bufs=2) as tmp_pool, \
             tc.tile_pool(name="psA", bufs=3, space="PSUM") as psA, \
             tc.tile_pool(name="psT", bufs=2, space="PSUM") as psT, \
             tc.tile_pool(name="psR", bufs=2, space="PSUM") as psR:
            part = [pp.tile([128, T], F32, name=f"part{m}") for m in range(6)]

            def load_w(kp, split=False):
                wt = wp.tile([128, KPQ * MW], F32R, tag="w", name=f"w{kp}")
                base = kp * KPQ * MW
                if split:
                    q_ = KPQ * MW // 4
                    for qi in range(4):
                        nc.sync.dma_start(wt[:, qi * q_:(qi + 1) * q_],
                                          wqkvp[:, base + qi * q_:base + (qi + 1) * q_])
                else:
                    nc.sync.dma_start(wt[:], wqkvp[:, base:base + KPQ * MW])
                return wt

            def load_hid(kp, ch, split=False):
                ht = hp.tile([128, KPQ * AC2], F32R, tag="hid", name=f"h{kp}_{ch}")
                base = (kp * NAC2 + ch) * KPQ * AC2
                if split:
                    q_ = KPQ * AC2 // 4
                    for qi in range(4):
                        nc.sync.dma_start(ht[:, qi * q_:(qi + 1) * q_],
                                          hidp[:, base + qi * q_:base + (qi + 1) * q_])
                else:
                    nc.sync.dma_start(ht[:], hidp[:, base:base + KPQ * AC2])
                return ht

            h_cur = load_hid(0, 0, split=True)
            w_tiles = {0: load_w(0, split=True)}
            nc.sync.dma_start(ones_sb[:], onesd[:, :])
            nc.sync.dma_start(ident[:], identd[:, :])
            nc.sync.dma_start(rmatT_sb[:], rmatT[:, :])
            w_tiles[1] = load_w(1)
            for kp in range(NKP):
                w_cur = w_tiles.pop(kp)
                for ch in range(NAC2):
                    asl = slice(ch * AC2, (ch + 1) * AC2)
                    if h_cur is None:
                        h_cur = load_hid(kp, ch)
                    if kp == 2 and ch == 0:
                        nc.sync.dma_start(cos_sb[:], cosT[:, :])
                        nc.sync.dma_start(sin_sb[:], sinT[:, :])
                    if ch == 1 and kp + 2 < NKP:
                        w_tiles[kp + 2] = load_w(kp + 2)
                    for m in range(6):
                        ps = psA.tile([128, AC2], F32, tag="psA")
                        for k in range(KPQ):
                            nc.tensor.matmul(
                                ps[:],
                                w_cur[:, k * MW + m * 128:k * MW + (m + 1) * 128],
                                h_cur[:, k * AC2:(k + 1) * AC2],
                                start=(k == 0), stop=(k == KPQ - 1))
                        if kp == 0:
                            nc.vector.tensor_copy(part[m][:, asl], ps[:])
                        elif kp < NKP - 1:
                            nc.vector.tensor_add(part[m][:, asl], part[m][:, asl],
                                                 ps[:])
                        else:
                            # final pass: single rounding into f32r qkvT
                            nc.vector.tensor_add(qk_t[m][:, asl], part[m][:, asl],
                                                 ps[:])
                            if m < 5:
                                # fused RoPE: x = x*cos + (rmatT.T@x)*sin
                                rp = psR.tile([128, AC2], F32, tag="psR")
                                nc.tensor.matmul(rp[:], rmatT_sb[:], qk_t[m][:, asl],
                                                 start=True, stop=True)
                                tmp = tmp_pool.tile([128, AC2], F32, tag="tmp")
                                nc.vector.tensor_mul(tmp[:], rp[:], sin_sb[:, asl])
                                nc.vector.tensor_mul(qk_t[m][:, asl],
                                                     qk_t[m][:, asl], cos_sb[:, asl])
                                nc.vector.tensor_add(qk_t[m][:, asl],
                                                     qk_t[m][:, asl], tmp[:])
                            else:
                                for jj in range(AC2 // 128):
                                    j = ch * (AC2 // 128) + jj
                                    pst = psT.tile([128, 128], F32R, tag="psT")
                                    nc.tensor.transpose(
                                        pst[:], qk_t[5][:, j * 128:(j + 1) * 128],
                                        ident[:])
                                    nc.vector.tensor_copy(v_sb[:, j, :], pst[:])
                    h_cur = None

        # ---------------- Phase B consts ----------------
        bconsts = ctx.enter_context(tc.tile_pool(name="bconsts", bufs=1))
        mask_sb = bconsts.tile([128, 4 * CH], F32)
        nc.sync.dma_start(mask_sb[:], masksp[:, :])

        wopool = ctx.enter_context(tc.tile_pool(name="wo", bufs=1))
        wo_sb = wopool.tile([128, HPC * (HID // 128) * 128], F32R)
        nc.sync.dma_start(wo_sb[:], wop[:, :])

        probs_pool = ctx.enter_context(tc.tile_pool(name="probs", bufs=5))
        attn_pool = ctx.enter_context(tc.tile_pool(name="attnT", bufs=1))
        attn_sb = [attn_pool.tile([128, T], F32R, name=f"attnT{h}") for h in range(HPC)]
        rden_pool = ctx.enter_context(tc.tile_pool(name="rden", bufs=2))
        out_pool = ctx.enter_context(tc.tile_pool(name="outstage", bufs=4))
        psS = ctx.enter_context(tc.tile_pool(name="psS", bufs=2, space="PSUM"))
        psAcc = ctx.enter_context(tc.tile_pool(name="psAcc", bufs=2, space="PSUM"))
        psD = ctx.enter_context(tc.tile_pool(name="psD", bufs=2, space="PSUM"))
        psO = ctx.enter_context(tc.tile_pool(name="psO", bufs=2, space="PSUM"))

        # ---------------- Phase B+C per tq chunk (C deferred one chunk) ---
        def phase_c(c):
            sl = slice(c * CH, (c + 1) * CH)
            for mo in range(HID // 128):
                po = psO.tile([128, CH], F32, tag="po")
                for kk in range(HPC):
                    woff = (kk * (HID // 128) + mo) * 128
                    nc.tensor.matmul(po[:], wo_sb[:, woff:woff + 128],
                                     attn_sb[kk][:, sl],
                                     start=(kk == 0), stop=(kk == HPC - 1))
                ob = out_pool.tile([128, CH], F32, tag="ob")
                if mo % 2 == 0:
                    nc.scalar.copy(ob[:], po[:])
                else:
                    nc.vector.tensor_copy(ob[:], po[:])
                nc.sync.dma_start(outT[mo * 128:(mo + 1) * 128, sl], ob[:])

        for c in range(NCH):
            sl = slice(c * CH, (c + 1) * CH)
            nts = (CH // 128) * (c + 1)  # causal: ts tiles 0..nts-1
            for h in range(HPC):
                pa = psAcc.tile([128, CH], F32, tag="acc")
                pd = psD.tile([128, CH], F32, tag="den")
                for j in range(nts):
                    i = j - (CH // 128) * c
                    # diagonal tiles: cols < i*128 fully masked; shrink to
                    # [off:512] (off capped at 256 to keep f32r N>=256 fast)
                    off = 0 if i < 1 else min(i * 128, 256)
                    csl = slice(c * CH + off, (c + 1) * CH)
                    ss = psS.tile([128, CH], F32, tag="psS")
                    nc.tensor.matmul(ss[:, off:], qk_t[4][:, j * 128:(j + 1) * 128],
                                     qk_t[h][:, csl], start=True, stop=True)
                    pr = probs_pool.tile([128, CH], F32R, tag="probs")
                    nc.scalar.activation(pr[:, off:], ss[:, off:], AF.Exp,
                                         scale=SCALING)
                    if i >= 0:
                        nc.vector.tensor_mul(pr[:, off:], pr[:, off:],
                                             mask_sb[:, i * CH + off:(i + 1) * CH])
                    nc.tensor.matmul(pd[:, off:], ones_sb[:], pr[:, off:],
                                     start=(j == 0), stop=(j == nts - 1))
                    nc.tensor.matmul(pa[:, off:], v_sb[:, j, :], pr[:, off:],
                                     start=(j == 0), stop=(j == nts - 1))
                rden = rden_pool.tile([128, CH], F32, tag="rden")
                nc.vector.reciprocal_approx_fast(out=rden[:], in_=pd[:])
                nc.vector.tensor_mul(attn_sb[h][:, sl], pa[:], rden[:])
            if c >= 1:
                phase_c(c - 1)
        phase_c(NCH - 1)

    nc.compile()
    return nc


def get_nc():
    if "nc" not in _CACHE:
        _CACHE["nc"] = _build_nc()
    return _CACHE["nc"]


def prep_in_maps(hidden_states, cos, sin, Wq, Wk, Wv, Wo):
    # hidp[p, (((kp*NAC2)+ch)*KPQ + ko)*AC2 + t] = hidT[(kp*KPQ+ko)*128+p, ch*AC2+t]
    hidT = hidden_states.T  # [HID, T]
    hidp = (hidT.reshape(NKP, KPQ, 128, NAC2, AC2)
            .transpose(2, 0, 3, 1, 4).reshape(128, -1))
    hidp = np.ascontiguousarray(hidp)
    cosT = np.ascontiguousarray(cos.T)
    sinT = np.ascontiguousarray(sin.T)
    # signed rotate-half permutation (as lhsT): rot = rmatT.T @ x
    rmatT = np.zeros((D, D), dtype=np.float32)
    half = D // 2
    rmatT[np.arange(half) + half, np.arange(half)] = -1.0  # rot[d<64] = -x[d+64]
    rmatT[np.arange(half), np.arange(half) + half] = 1.0   # rot[d>=64] = x[d-64]
    # masksp[p, i*CH+f] = 1 if i*128+p <= f
    i_idx = np.arange(4)[None, :, None] * 128
    p_idx = np.arange(128)[:, None, None]
    f_idx = np.arange(CH)[None, None, :]
    masksp = ((i_idx + p_idx) <= f_idx).astype(np.float32).reshape(128, -1)

    in_maps = []
    for c in range(NCORES):
        wqkv = np.concatenate([
            Wq[:, c * QW:(c + 1) * QW],
            Wk[:, c * D:(c + 1) * D],
            Wv[:, c * D:(c + 1) * D],
        ], axis=1)  # [HID, MW]
        # wqkvp[p, ko*MW+m] = wqkv[ko*128+p, m]
        wqkvp = np.ascontiguousarray(
            wqkv.reshape(KO, 128, MW).transpose(1, 0, 2).reshape(128, -1))
        # wop[p, (kk*(HID//128)+mo)*128+q] = Wo[c*QW + kk*128+p, mo*128+q]
        wo_c = Wo[c * QW:(c + 1) * QW, :]
        wop_ = np.ascontiguousarray(
            wo_c.reshape(HPC, 128, HID // 128, 128).transpose(1, 0, 2, 3)
            .reshape(128, -1))
        in_maps.append({
            "hidp": hidp,
            "wqkvp": wqkvp,
            "wop": wop_,
            "cosT": cosT,
            "sinT": sinT,
            "rmatT": rmatT,
            "onesd": np.ones((128, 128), dtype=np.float32),
            "identd": np.eye(128, dtype=np.float32),
            "masksp": masksp,
        })
    return in_maps


def postprocess(results):
    acc = results[0]["outT_p"].copy()
    for r in results[1:]:
        acc += r["outT_p"]
    return np.ascontiguousarray(acc.T).astype(np.float32)


def kernel(hidden_states, position_ids, cos, sin, Wq, Wk, Wv, Wo):
    from concourse.bass_utils import run_bass_kernel_spmd
    nc = get_nc()
    in_maps = prep_in_maps(hidden_states, cos, sin, Wq, Wk, Wv, Wo)
    res = run_bass_kernel_spmd(nc, in_maps, core_ids=list(range(NCORES)))
    return postprocess(res.results)


# revision 20
# speedup vs baseline: 1.1570x; 1.0072x over previous
"""Trainium2 Bass kernel for nn_LlamaAttention (T=2048, HID=4096, HQ=32, HKV=8, D=128).

Tensor-parallel over heads across 8 NeuronCores: core c owns q-heads 4c..4c+3 and
kv-head c (GQA group size 4 == heads-per-core, so attention is fully core-local).
Wo is row-sharded; each core computes a partial [T, HID] output (transposed) and
the host sums the 8 partials. No device collectives.

All weight/activation inputs are host-pre-swizzled to partition-major layouts so
every DMA is one contiguous run per partition (descriptor-generation on the Sync
engine is the startup bottleneck otherwise).

Self-contained: hardcodes all shapes; builds the Bass kernel once per process.
"""
import numpy as np

T, HID, HQ, HKV, D = 2048, 4096, 32, 8, 128
NCORES = 8
HPC = HQ // NCORES            # 4 q heads per core
QW = HPC * D                  # 512 q columns per core
MW = QW + 2 * D               # 768 qkv columns per core
KO = HID // 128               # 32 k-tiles
NKP = 4                       # phase A K passes
KPQ = KO // NKP               # 8 k-tiles per pass
AC2 = 512                     # phase A T-chunk width
NAC2 = T // AC2               # 4
CH = 512                      # attention tq chunk width
NCH = T // CH                 # 4
SCALING = float(D) ** -0.5

_CACHE = {}


def _build_nc():
    import concourse.mybir as mybir
    import concourse.tile as tile
    from concourse import bacc
    from concourse.bass import _add_dep_helper
    from contextlib import ExitStack

    F32 = mybir.dt.float32
    F32R = mybir.dt.float32r
    AF = mybir.ActivationFunctionType

    nc = bacc.Bacc("TRN2", target_bir_lowering=False, debug=False,
                   dynamic_dma_scratch_size=2048)

    # pre-swizzled inputs (see prep_in_maps)
    hidp = nc.dram_tensor("hidp", [128, NKP * NAC2 * KPQ * AC2], F32R,
                          kind="ExternalInput")
    wqkvp = nc.dram_tensor("wqkvp", [128, KO * MW], F32R, kind="ExternalInput")
    wop = nc.dram_tensor("wop", [128, HPC * (HID // 128) * 128], F32R,
                         kind="ExternalInput")
    cosT = nc.dram_tensor("cosT", [D, T], F32, kind="ExternalInput")
    sinT = nc.dram_tensor("sinT", [D, T], F32, kind="ExternalInput")
    rmatT = nc.dram_tensor("rmatT", [D, D], F32R, kind="ExternalInput")
    onesd = nc.dram_tensor("onesd", [128, 128], F32R, kind="ExternalInput")
    identd = nc.dram_tensor("identd", [128, 128], F32R, kind="ExternalInput")
    masksp = nc.dram_tensor("masksp", [128, 4 * CH], F32, kind="ExternalInput")
    outT = nc.dram_tensor("outT_p", [HID, T], F32, kind="ExternalOutput")

    with tile.TileContext(nc) as tc, ExitStack() as ctx:
        consts = ctx.enter_context(tc.tile_pool(name="consts", bufs=1))
        ones_sb = consts.tile([128, 128], F32R)
        ident = consts.tile([128, 128], F32R)
        rmatT_sb = consts.tile([128, 128], F32R)

        qkv_pool = ctx.enter_context(tc.tile_pool(name="qkvT", bufs=1))
        # m=0..3 q heads, m=4 kT, m=5 vT
        qk_t = [qkv_pool.tile([128, T], F32R, name=f"qkvT{m}") for m in range(6)]
        v_sb = qkv_pool.tile([128, T // 128, 128], F32R, name="v_sb")  # [ts, blk, D]

        csp = ctx.enter_context(tc.tile_pool(name="cs", bufs=1))
        cos_sb = csp.tile([128, T], F32)
        sin_sb = csp.tile([128, T], F32)

        # ---- Phase A: qkvT = wqkv.T @ hidT in NKP K-passes, fp32 partials ----
        with tc.tile_pool(name="w", bufs=2) as wp, \
             tc.tile_pool(name="hid", bufs=3) as hp, \
             tc.tile_pool(name="partial", bufs=1) as pp, \
             tc.tile_pool(name="ropetmp", bufs=2) as tmp_pool, \
             tc.tile_pool(name="psA", bufs=3, space="PSUM") as psA, \
             tc.tile_pool(name="psT", bufs=2, space="PSUM") as psT, \
             tc.tile_pool(name="psR", bufs=2, space="PSUM") as psR:
            part = [pp.tile([128, T], F32, name=f"part{m}") for m in range(6)]

            def load_w(kp, split=False):
                wt = wp.tile([128, KPQ * MW], F32R, tag="w", name=f"w{kp}")
                base = kp * KPQ * MW
                if split:
                    q_ = KPQ * MW // 4
                    for qi in range(4):
                        nc.sync.dma_start(wt[:, qi * q_:(qi + 1) * q_],
                                          wqkvp[:, base + qi * q_:base + (qi + 1) * q_])
                else:
                    nc.sync.dma_start(wt[:], wqkvp[:, base:base + KPQ * MW])
                return wt

            def load_hid(kp, ch, split=False):
                ht = hp.tile([128, KPQ * AC2], F32R, tag="hid", name=f"h{kp}_{ch}")
                base = (kp * NAC2 + ch) * KPQ * AC2
                if split:
                    q_ = KPQ * AC2 // 4
                    for qi in range(4):
                        nc.sync.dma_start(ht[:, qi * q_:(qi + 1) * q_],
                                          hidp[:, base + qi * q_:base + (qi + 1) * q_])
                else:
                    nc.sync.dma_start(ht[:], hidp[:, base:base + KPQ * AC2])
                return ht

            # startup: only the first hid/w quarters go out alone; every
            # other initial DMA waits on them so they get full bandwidth
            # (HWDGE queues fair-share bandwidth across in-flight DMAs).
            first_dmas = []
            ht0 = hp.tile([128, KPQ * AC2], F32R, tag="hid", name="h0_0")
            q_ = KPQ * AC2 // 4
            first_dmas.append(nc.sync.dma_start(ht0[:, :q_], hidp[:, 0:q_]))
            wt0 = wp.tile([128, KPQ * MW], F32R, tag="w", name="w0")
            qw_ = KPQ * MW // 4
            first_dmas.append(nc.sync.dma_start(wt0[:, :qw_], wqkvp[:, 0:qw_]))

            def defer(bi):
                for f in first_dmas:
                    _add_dep_helper(bi.ins, f.ins, sync=True,
                                    reason="defer bulk DMA behind first tiles")
                return bi

            for qi in range(1, 4):
                defer(nc.sync.dma_start(ht0[:, qi * q_:(qi + 1) * q_],
                                        hidp[:, qi * q_:(qi + 1) * q_]))
                defer(nc.sync.dma_start(wt0[:, qi * qw_:(qi + 1) * qw_],
                                        wqkvp[:, qi * qw_:(qi + 1) * qw_]))
            h_cur = ht0
            w_tiles = {0: wt0}
            defer(nc.sync.dma_start(ones_sb[:], onesd[:, :]))
            defer(nc.sync.dma_start(ident[:], identd[:, :]))
            defer(nc.sync.dma_start(rmatT_sb[:], rmatT[:, :]))
            w_tiles[1] = load_w(1)
            for kp in range(NKP):
                w_cur = w_tiles.pop(kp)
                for ch in range(NAC2):
                    asl = slice(ch * AC2, (ch + 1) * AC2)
                    if h_cur is None:
                        h_cur = load_hid(kp, ch)
                    if kp == 2 and ch == 0:
                        nc.sync.dma_start(cos_sb[:], cosT[:, :])
                        nc.sync.dma_start(sin_sb[:], sinT[:, :])
                    if ch == 1 and kp + 2 < NKP:
                        w_tiles[kp + 2] = load_w(kp + 2)
                    for m in range(6):
                        ps = psA.tile([128, AC2], F32, tag="psA")
                        for k in range(KPQ):
                            nc.tensor.matmul(
                                ps[:],
                                w_cur[:, k * MW + m * 128:k * MW + (m + 1) * 128],
                                h_cur[:, k * AC2:(k + 1) * AC2],
                                start=(k == 0), stop=(k == KPQ - 1))
                        if kp == 0:
                            nc.vector.tensor_copy(part[m][:, asl], ps[:])
                        elif kp < NKP - 1:
                            nc.vector.tensor_add(part[m][:, asl], part[m][:, asl],
                                                 ps[:])
                        else:
                            # final pass: single rounding into f32r qkvT
                            nc.vector.tensor_add(qk_t[m][:, asl], part[m][:, asl],
                                                 ps[:])
                            if m < 5:
                                # fused RoPE: x = x*cos + (rmatT.T@x)*sin
                                rp = psR.tile([128, AC2], F32, tag="psR")
                                nc.tensor.matmul(rp[:], rmatT_sb[:], qk_t[m][:, asl],
                                                 start=True, stop=True)
                                tmp = tmp_pool.tile([128, AC2], F32, tag="tmp")
                                nc.vector.tensor_mul(tmp[:], rp[:], sin_sb[:, asl])
                                nc.vector.tensor_mul(qk_t[m][:, asl],
                                                     qk_t[m][:, asl], cos_sb[:, asl])
                                nc.vector.tensor_add(qk_t[m][:, asl],
                                                     qk_t[m][:, asl], tmp[:])
                            else:
                                for jj in range(AC2 // 128):
                                    j = ch * (AC2 // 128) + jj
                                    pst = psT.tile([128, 128], F32R, tag="psT")
                                    nc.tensor.transpose(
                                        pst[:], qk_t[5][:, j * 128:(j + 1) * 128],
                                        ident[:])
                                    nc.vector.tensor_copy(v_sb[:, j, :], pst[:])
                    h_cur = None

        # ---------------- Phase B consts ----------------
        bconsts = ctx.enter_context(tc.tile_pool(name="bconsts", bufs=1))
        mask_sb = bconsts.tile([128, 4 * CH], F32)
        nc.sync.dma_start(mask_sb[:], masksp[:, :])

        wopool = ctx.enter_context(tc.tile_pool(name="wo", bufs=1))
        wo_sb = wopool.tile([128, HPC * (HID // 128) * 128], F32R)
        nc.sync.dma_start(wo_sb[:], wop[:, :])

        probs_pool = ctx.enter_context(tc.tile_pool(name="probs", bufs=5))
        attn_pool = ctx.enter_context(tc.tile_pool(name="attnT", bufs=1))
        attn_sb = [attn_pool.tile([128, T], F32R, name=f"attnT{h}") for h in range(HPC)]
        rden_pool = ctx.enter_context(tc.tile_pool(name="rden", bufs=2))
        out_pool = ctx.enter_context(tc.tile_pool(name="outstage", bufs=4))
        psS = ctx.enter_context(tc.tile_pool(name="psS", bufs=2, space="PSUM"))
        psAcc = ctx.enter_context(tc.tile_pool(name="psAcc", bufs=2, space="PSUM"))
        psD = ctx.enter_context(tc.tile_pool(name="psD", bufs=2, space="PSUM"))
        psO = ctx.enter_context(tc.tile_pool(name="psO", bufs=2, space="PSUM"))

        # ---------------- Phase B+C per tq chunk (C deferred one chunk) ---
        def phase_c(c):
            sl = slice(c * CH, (c + 1) * CH)
            for mo in range(HID // 128):
                po = psO.tile([128, CH], F32, tag="po")
                for kk in range(HPC):
                    woff = (kk * (HID // 128) + mo) * 128
                    nc.tensor.matmul(po[:], wo_sb[:, woff:woff + 128],
                                     attn_sb[kk][:, sl],
                                     start=(kk == 0), stop=(kk == HPC - 1))
                ob = out_pool.tile([128, CH], F32, tag="ob")
                if mo % 2 == 0:
                    nc.scalar.copy(ob[:], po[:])
                else:
                    nc.vector.tensor_copy(ob[:], po[:])
                nc.sync.dma_start(outT[mo * 128:(mo + 1) * 128, sl], ob[:])

        for c in range(NCH):
            sl = slice(c * CH, (c + 1) * CH)
            nts = (CH // 128) * (c + 1)  # causal: ts tiles 0..nts-1
            for h in range(HPC):
                pa = psAcc.tile([128, CH], F32, tag="acc")
                pd = psD.tile([128, CH], F32, tag="den")
                for j in range(nts):
                    i = j - (CH // 128) * c
                    # diagonal tiles: cols < i*128 fully masked; shrink to
                    # [off:512] (off capped at 256 to keep f32r N>=256 fast)
                    off = 0 if i < 1 else min(i * 128, 256)
                    csl = slice(c * CH + off, (c + 1) * CH)
                    ss = psS.tile([128, CH], F32, tag="psS")
                    nc.tensor.matmul(ss[:, off:], qk_t[4][:, j * 128:(j + 1) * 128],
                                     qk_t[h][:, csl], start=True, stop=True)
                    pr = probs_pool.tile([128, CH], F32R, tag="probs")
                    nc.scalar.activation(pr[:, off:], ss[:, off:], AF.Exp,
                                         scale=SCALING)
                    if i >= 0:
                        nc.vector.tensor_mul(pr[:, off:], pr[:, off:],
                                             mask_sb[:, i * CH + off:(i + 1) * CH])
                    nc.tensor.matmul(pd[:, off:], ones_sb[:], pr[:, off:],
                                     start=(j == 0), stop=(j == nts - 1))
                    nc.tensor.matmul(pa[:, off:], v_sb[:, j, :], pr[:, off:],
                                     start=(j == 0), stop=(j == nts - 1))
                rden = rden_pool.tile([128, CH], F32, tag="rden")
                nc.vector.reciprocal_approx_fast(out=rden[:], in_=pd[:])
                nc.vector.tensor_mul(attn_sb[h][:, sl], pa[:], rden[:])
            if c >= 1:
                phase_c(c - 1)
        phase_c(NCH - 1)

    nc.compile()
    return nc


def get_nc():
    if "nc" not in _CACHE:
        _CACHE["nc"] = _build_nc()
    return _CACHE["nc"]


def prep_in_maps(hidden_states, cos, sin, Wq, Wk, Wv, Wo):
    # hidp[p, (((kp*NAC2)+ch)*KPQ + ko)*AC2 + t] = hidT[(kp*KPQ+ko)*128+p, ch*AC2+t]
    hidT = hidden_states.T  # [HID, T]
    hidp = (hidT.reshape(NKP, KPQ, 128, NAC2, AC2)
            .transpose(2, 0, 3, 1, 4).reshape(128, -1))
    hidp = np.ascontiguousarray(hidp)
    cosT = np.ascontiguousarray(cos.T)
    sinT = np.ascontiguousarray(sin.T)
    # signed rotate-half permutation (as lhsT): rot = rmatT.T @ x
    rmatT = np.zeros((D, D), dtype=np.float32)
    half = D // 2
    rmatT[np.arange(half) + half, np.arange(half)] = -1.0  # rot[d<64] = -x[d+64]
    rmatT[np.arange(half), np.arange(half) + half] = 1.0   # rot[d>=64] = x[d-64]
    # masksp[p, i*CH+f] = 1 if i*128+p <= f
    i_idx = np.arange(4)[None, :, None] * 128
    p_idx = np.arange(128)[:, None, None]
    f_idx = np.arange(CH)[None, None, :]
    masksp = ((i_idx + p_idx) <= f_idx).astype(np.float32).reshape(128, -1)

    in_maps = []
    for c in range(NCORES):
        wqkv = np.concatenate([
            Wq[:, c * QW:(c + 1) * QW],
            Wk[:, c * D:(c + 1) * D],
            Wv[:, c * D:(c + 1) * D],
        ], axis=1)  # [HID, MW]
        # wqkvp[p, ko*MW+m] = wqkv[ko*128+p, m]
        wqkvp = np.ascontiguousarray(
            wqkv.reshape(KO, 128, MW).transpose(1, 0, 2).reshape(128, -1))
        # wop[p, (kk*(HID//128)+mo)*128+q] = Wo[c*QW + kk*128+p, mo*128+q]
        wo_c = Wo[c * QW:(c + 1) * QW, :]
        wop_ = np.ascontiguousarray(
            wo_c.reshape(HPC, 128, HID // 128, 128).transpose(1, 0, 2, 3)
            .reshape(128, -1))
        in_maps.append({
            "hidp": hidp,
            "wqkvp": wqkvp,
            "wop": wop_,
            "cosT": cosT,
            "sinT": sinT,
            "rmatT": rmatT,
            "onesd": np.ones((128, 128), dtype=np.float32),
            "identd": np.eye(128, dtype=np.float32),
            "masksp": masksp,
        })
    return in_maps


def postprocess(results):
    acc = results[0]["outT_p"].copy()
    for r in results[1:]:
        acc += r["outT_p"]
    return np.ascontiguousarray(acc.T).astype(np.float32)


def kernel(hidden_states, position_ids, cos, sin, Wq, Wk, Wv, Wo):
    from concourse.bass_utils import run_bass_kernel_spmd
    nc = get_nc()
    in_maps = prep_in_maps(hidden_states, cos, sin, Wq, Wk, Wv, Wo)
    res = run_bass_kernel_spmd(nc, in_maps, core_ids=list(range(NCORES)))
    return postprocess(res.results)


# revision 21
# speedup vs baseline: 1.1925x; 1.0307x over previous
"""Trainium2 Bass kernel for nn_LlamaAttention (T=2048, HID=4096, HQ=32, HKV=8, D=128).

Tensor-parallel over heads across 8 NeuronCores: core c owns q-heads 4c..4c+3 and
kv-head c (GQA group size 4 == heads-per-core, so attention is fully core-local).
Wo is row-sharded; each core computes a partial [T, HID] output (transposed) and
the host sums the 8 partials. No device collectives.

All weight/activation inputs are host-pre-swizzled to partition-major layouts so
every DMA is one contiguous run per partition (descriptor-generation on the Sync
engine is the startup bottleneck otherwise).

Self-contained: hardcodes all shapes; builds the Bass kernel once per process.
"""
import numpy as np

T, HID, HQ, HKV, D = 2048, 4096, 32, 8, 128
NCORES = 8
HPC = HQ // NCORES            # 4 q heads per core
QW = HPC * D                  # 512 q columns per core
MW = QW + 2 * D               # 768 qkv columns per core
KO = HID // 128               # 32 k-tiles
NKP = 4                       # phase A K passes
KPQ = KO // NKP               # 8 k-tiles per pass
AC2 = 512                     # phase A T-chunk width
NAC2 = T // AC2               # 4
CH = 512                      # attention tq chunk width
NCH = T // CH                 # 4
SCALING = float(D) ** -0.5

_CACHE = {}


def _build_nc():
    import concourse.mybir as mybir
    import concourse.tile as tile
    from concourse import bacc
    from concourse.bass import _add_dep_helper
    from contextlib import ExitStack

    F32 = mybir.dt.float32
    F32R = mybir.dt.float32r
    AF = mybir.ActivationFunctionType

    nc = bacc.Bacc("TRN2", target_bir_lowering=False, debug=False,
                   dynamic_dma_scratch_size=2048)

    # pre-swizzled inputs (see prep_in_maps)
    hidp = nc.dram_tensor("hidp", [128, NKP * NAC2 * KPQ * AC2], F32R,
                          kind="ExternalInput")
    wqkvp = nc.dram_tensor("wqkvp", [128, KO * MW], F32R, kind="ExternalInput")
    wop = nc.dram_tensor("wop", [128, HPC * (HID // 128) * 128], F32R,
                         kind="ExternalInput")
    cosT = nc.dram_tensor("cosT", [D, T], F32, kind="ExternalInput")
    sinT = nc.dram_tensor("sinT", [D, T], F32, kind="ExternalInput")
    rmatT = nc.dram_tensor("rmatT", [D, D], F32R, kind="ExternalInput")
    onesd = nc.dram_tensor("onesd", [128, 128], F32R, kind="ExternalInput")
    identd = nc.dram_tensor("identd", [128, 128], F32R, kind="ExternalInput")
    masksp = nc.dram_tensor("masksp", [128, 4 * CH], F32, kind="ExternalInput")
    outT = nc.dram_tensor("outT_p", [HID, T], F32, kind="ExternalOutput")

    with tile.TileContext(nc) as tc, ExitStack() as ctx:
        consts = ctx.enter_context(tc.tile_pool(name="consts", bufs=1))
        ones_sb = consts.tile([128, 128], F32R)
        ident = consts.tile([128, 128], F32R)
        rmatT_sb = consts.tile([128, 128], F32R)

        qkv_pool = ctx.enter_context(tc.tile_pool(name="qkvT", bufs=1))
        # m=0..3 q heads, m=4 kT, m=5 vT
        qk_t = [qkv_pool.tile([128, T], F32R, name=f"qkvT{m}") for m in range(6)]
        v_sb = qkv_pool.tile([128, T // 128, 128], F32R, name="v_sb")  # [ts, blk, D]

        csp = ctx.enter_context(tc.tile_pool(name="cs", bufs=1))
        cos_sb = csp.tile([128, T], F32)
        sin_sb = csp.tile([128, T], F32)

        # ---- Phase A: qkvT = wqkv.T @ hidT in NKP K-passes, fp32 partials ----
        with tc.tile_pool(name="w", bufs=2) as wp, \
             tc.tile_pool(name="hid", bufs=3) as hp, \
             tc.tile_pool(name="partial", bufs=1) as pp, \
             tc.tile_pool(name="ropetmp", bufs=2) as tmp_pool, \
             tc.tile_pool(name="psA", bufs=3, space="PSUM") as psA, \
             tc.tile_pool(name="psT", bufs=2, space="PSUM") as psT, \
             tc.tile_pool(name="psR", bufs=2, space="PSUM") as psR:
            part = [pp.tile([128, T], F32, name=f"part{m}") for m in range(6)]

            def load_w(kp, split=False):
                wt = wp.tile([128, KPQ * MW], F32R, tag="w", name=f"w{kp}")
                base = kp * KPQ * MW
                if split:
                    q_ = KPQ * MW // 4
                    for qi in range(4):
                        nc.sync.dma_start(wt[:, qi * q_:(qi + 1) * q_],
                                          wqkvp[:, base + qi * q_:base + (qi + 1) * q_])
                else:
                    nc.sync.dma_start(wt[:], wqkvp[:, base:base + KPQ * MW])
                return wt

            def load_hid(kp, ch, split=False):
                ht = hp.tile([128, KPQ * AC2], F32R, tag="hid", name=f"h{kp}_{ch}")
                base = (kp * NAC2 + ch) * KPQ * AC2
                if split:
                    q_ = KPQ * AC2 // 4
                    for qi in range(4):
                        nc.sync.dma_start(ht[:, qi * q_:(qi + 1) * q_],
                                          hidp[:, base + qi * q_:base + (qi + 1) * q_])
                else:
                    nc.sync.dma_start(ht[:], hidp[:, base:base + KPQ * AC2])
                return ht

            # startup: only the first hid/w quarters go out alone; every
            # other initial DMA waits on them so they get full bandwidth
            # (HWDGE queues fair-share bandwidth across in-flight DMAs).
            first_dmas = []
            ht0 = hp.tile([128, KPQ * AC2], F32R, tag="hid", name="h0_0")
            q_ = KPQ * AC2 // 4
            first_dmas.append(nc.sync.dma_start(ht0[:, :q_], hidp[:, 0:q_]))
            wt0 = wp.tile([128, KPQ * MW], F32R, tag="w", name="w0")
            qw_ = KPQ * MW // 4
            first_dmas.append(nc.sync.dma_start(wt0[:, :qw_], wqkvp[:, 0:qw_]))

            def defer(bi):
                for f in first_dmas:
                    _add_dep_helper(bi.ins, f.ins, sync=True,
                                    reason="defer bulk DMA behind first tiles")
                return bi

            for qi in range(1, 4):
                defer(nc.sync.dma_start(ht0[:, qi * q_:(qi + 1) * q_],
                                        hidp[:, qi * q_:(qi + 1) * q_]))
                defer(nc.sync.dma_start(wt0[:, qi * qw_:(qi + 1) * qw_],
                                        wqkvp[:, qi * qw_:(qi + 1) * qw_]))
            h_cur = ht0
            w_tiles = {0: wt0}
            defer(nc.sync.dma_start(ones_sb[:], onesd[:, :]))
            defer(nc.sync.dma_start(ident[:], identd[:, :]))
            defer(nc.sync.dma_start(rmatT_sb[:], rmatT[:, :]))
            w_tiles[1] = load_w(1)
            for kp in range(NKP):
                w_cur = w_tiles.pop(kp)
                for ch in range(NAC2):
                    asl = slice(ch * AC2, (ch + 1) * AC2)
                    if h_cur is None:
                        h_cur = load_hid(kp, ch)
                    if kp == 2 and ch == 0:
                        nc.sync.dma_start(cos_sb[:], cosT[:, :])
                        nc.sync.dma_start(sin_sb[:], sinT[:, :])
                    if ch == 1 and kp + 2 < NKP:
                        w_tiles[kp + 2] = load_w(kp + 2)
                    for m in range(6):
                        ps = psA.tile([128, AC2], F32, tag="psA")
                        for k in range(KPQ):
                            nc.tensor.matmul(
                                ps[:],
                                w_cur[:, k * MW + m * 128:k * MW + (m + 1) * 128],
                                h_cur[:, k * AC2:(k + 1) * AC2],
                                start=(k == 0), stop=(k == KPQ - 1))
                        if kp == 0:
                            nc.vector.tensor_copy(part[m][:, asl], ps[:])
                        elif kp < NKP - 1:
                            nc.vector.tensor_add(part[m][:, asl], part[m][:, asl],
                                                 ps[:])
                        else:
                            # final pass: single rounding into f32r qkvT
                            nc.vector.tensor_add(qk_t[m][:, asl], part[m][:, asl],
                                                 ps[:])
                            if m < 5:
                                # fused RoPE: x = x*cos + (rmatT.T@x)*sin
                                rp = psR.tile([128, AC2], F32, tag="psR")
                                nc.tensor.matmul(rp[:], rmatT_sb[:], qk_t[m][:, asl],
                                                 start=True, stop=True)
                                tmp = tmp_pool.tile([128, AC2], F32, tag="tmp")
                                nc.vector.tensor_mul(tmp[:], rp[:], sin_sb[:, asl])
                                nc.vector.tensor_mul(qk_t[m][:, asl],
                                                     qk_t[m][:, asl], cos_sb[:, asl])
                                nc.vector.tensor_add(qk_t[m][:, asl],
                                                     qk_t[m][:, asl], tmp[:])
                            else:
                                for jj in range(AC2 // 128):
                                    j = ch * (AC2 // 128) + jj
                                    pst = psT.tile([128, 128], F32R, tag="psT")
                                    nc.tensor.transpose(
                                        pst[:], qk_t[5][:, j * 128:(j + 1) * 128],
                                        ident[:])
                                    nc.vector.tensor_copy(v_sb[:, j, :], pst[:])
                    h_cur = None

        # ---------------- Phase B consts ----------------
        bconsts = ctx.enter_context(tc.tile_pool(name="bconsts", bufs=1))
        mask_sb = bconsts.tile([128, 4 * CH], F32)
        nc.sync.dma_start(mask_sb[:], masksp[:, :])

        wopool = ctx.enter_context(tc.tile_pool(name="wo", bufs=1))
        wo_sb = wopool.tile([128, HPC * (HID // 128) * 128], F32R)
        nc.sync.dma_start(wo_sb[:], wop[:, :])

        probs_pool = ctx.enter_context(tc.tile_pool(name="probs", bufs=5))
        attn_pool = ctx.enter_context(tc.tile_pool(name="attnT", bufs=1))
        attn_sb = [attn_pool.tile([128, T], F32R, name=f"attnT{h}") for h in range(HPC)]
        rden_pool = ctx.enter_context(tc.tile_pool(name="rden", bufs=2))
        out_pool = ctx.enter_context(tc.tile_pool(name="outstage", bufs=4))
        psS = ctx.enter_context(tc.tile_pool(name="psS", bufs=2, space="PSUM"))
        psAcc = ctx.enter_context(tc.tile_pool(name="psAcc", bufs=2, space="PSUM"))
        psD = ctx.enter_context(tc.tile_pool(name="psD", bufs=2, space="PSUM"))
        psO = ctx.enter_context(tc.tile_pool(name="psO", bufs=2, space="PSUM"))

        # ---------------- Phase B+C per tq chunk (C deferred one chunk) ---
        def phase_c(c):
            sl = slice(c * CH, (c + 1) * CH)
            for mo in range(HID // 128):
                po = psO.tile([128, CH], F32, tag="po")
                for kk in range(HPC):
                    woff = (kk * (HID // 128) + mo) * 128
                    nc.tensor.matmul(po[:], wo_sb[:, woff:woff + 128],
                                     attn_sb[kk][:, sl],
                                     start=(kk == 0), stop=(kk == HPC - 1))
                ob = out_pool.tile([128, CH], F32, tag="ob")
                if mo % 2 == 0:
                    nc.scalar.copy(ob[:], po[:])
                else:
                    nc.vector.tensor_copy(ob[:], po[:])
                nc.sync.dma_start(outT[mo * 128:(mo + 1) * 128, sl], ob[:])

        for c in range(NCH):
            sl = slice(c * CH, (c + 1) * CH)
            nts = (CH // 128) * (c + 1)  # causal: ts tiles 0..nts-1
            for h in range(HPC):
                pa = psAcc.tile([128, CH], F32, tag="acc")
                pd = psD.tile([128, CH], F32, tag="den")
                for j in range(nts):
                    i = j - (CH // 128) * c
                    # diagonal tiles: cols < i*128 fully masked; shrink to
                    # [off:512] (off capped at 256 to keep f32r N>=256 fast)
                    off = 0 if i < 1 else min(i * 128, 256)
                    csl = slice(c * CH + off, (c + 1) * CH)
                    ss = psS.tile([128, CH], F32, tag="psS")
                    nc.tensor.matmul(ss[:, off:], qk_t[4][:, j * 128:(j + 1) * 128],
                                     qk_t[h][:, csl], start=True, stop=True)
                    pr = probs_pool.tile([128, CH], F32R, tag="probs")
                    nc.scalar.activation(pr[:, off:], ss[:, off:], AF.Exp,
                                         scale=SCALING)
                    if i >= 0:
                        nc.vector.tensor_mul(pr[:, off:], pr[:, off:],
                                             mask_sb[:, i * CH + off:(i + 1) * CH])
                    nc.tensor.matmul(pd[:, off:], ones_sb[:], pr[:, off:],
                                     start=(j == 0), stop=(j == nts - 1))
                    nc.tensor.matmul(pa[:, off:], v_sb[:, j, :], pr[:, off:],
                                     start=(j == 0), stop=(j == nts - 1))
                rden = rden_pool.tile([128, CH], F32, tag="rden")
                nc.vector.reciprocal_approx_fast(out=rden[:], in_=pd[:])
                nc.vector.tensor_mul(attn_sb[h][:, sl], pa[:], rden[:])
            if c >= 1:
                phase_c(c - 1)
        phase_c(NCH - 1)

    nc.compile()
    return nc


def get_nc():
    if "nc" not in _CACHE:
        _CACHE["nc"] = _build_nc()
    return _CACHE["nc"]


def prep_in_maps(hidden_states, cos, sin, Wq, Wk, Wv, Wo):
    # hidp[p, (((kp*NAC2)+ch)*KPQ + ko)*AC2 + t] = hidT[(kp*KPQ+ko)*128+p, ch*AC2+t]
    hidT = np.asarray(hidden_states).T  # [HID, T]
    hidp = (np.asarray(hidT).reshape(HID, T)
            .reshape(NKP, KPQ, 128, NAC2, AC2)
            .transpose(2, 0, 3, 1, 4).reshape(128, -1))
    hidp = np.ascontiguousarray(hidp)
    cosT = np.ascontiguousarray(np.asarray(cos).T)
    sinT = np.ascontiguousarray(np.asarray(sin).T)
    # signed rotate-half permutation (as lhsT): rot = rmatT.T @ x
    rmatT = np.zeros((D, D), dtype=np.float32)
    half = D // 2
    rmatT[np.arange(half) + half, np.arange(half)] = -1.0  # rot[d<64] = -x[d+64]
    rmatT[np.arange(half), np.arange(half) + half] = 1.0   # rot[d>=64] = x[d-64]
    # masksp[p, i*CH+f] = 1 if i*128+p <= f
    i_idx = np.arange(4)[None, :, None] * 128
    p_idx = np.arange(128)[:, None, None]
    f_idx = np.arange(CH)[None, None, :]
    masksp = ((i_idx + p_idx) <= f_idx).astype(np.float32).reshape(128, -1)

    in_maps = []
    for c in range(NCORES):
        wqkv = np.concatenate([
            Wq[:, c * QW:(c + 1) * QW],
            Wk[:, c * D:(c + 1) * D],
            Wv[:, c * D:(c + 1) * D],
        ], axis=1)  # [HID, MW]
        # wqkvp[p, ko*MW+m] = wqkv[ko*128+p, m]
        wqkvp = np.ascontiguousarray(
            wqkv.reshape(KO, 128, MW).transpose(1, 0, 2).reshape(128, -1))
        # wop[p, (kk*(HID//128)+mo)*128+q] = Wo[c*QW + kk*128+p, mo*128+q]
        wo_c = Wo[c * QW:(c + 1) * QW, :]
        wop_ = np.ascontiguousarray(
            wo_c.reshape(HPC, 128, HID // 128, 128).transpose(1, 0, 2, 3)
            .reshape(128, -1))
        in_maps.append({
            "hidp": hidp,
            "wqkvp": wqkvp,
            "wop": wop_,
            "cosT": cosT,
            "sinT": sinT,
            "rmatT": rmatT,
            "onesd": np.ones((128, 128), dtype=np.float32),
            "identd": np.eye(128, dtype=np.float32),
            "masksp": masksp,
        })
    return in_maps


def postprocess(results):
    acc = results[0]["outT_p"].copy()
    for r in results[1:]:
        acc += r["outT_p"]
    return np.ascontiguousarray(acc.T).astype(np.float32)


def kernel(hidden_states, position_ids, cos, sin, Wq, Wk, Wv, Wo):
    from concourse.bass_utils import run_bass_kernel_spmd
    hidden_states = np.asarray(hidden_states, dtype=np.float32)
    cos = np.asarray(cos, dtype=np.float32)
    sin = np.asarray(sin, dtype=np.float32)
    Wq = np.asarray(Wq, dtype=np.float32)
    Wk = np.asarray(Wk, dtype=np.float32)
    Wv = np.asarray(Wv, dtype=np.float32)
    Wo = np.asarray(Wo, dtype=np.float32)
    nc = get_nc()
    in_maps = prep_in_maps(hidden_states, cos, sin, Wq, Wk, Wv, Wo)
    res = run_bass_kernel_spmd(nc, in_maps, core_ids=list(range(NCORES)))
    return postprocess(res.results)


# revision 22
# speedup vs baseline: 1.2033x; 1.0091x over previous
"""Trainium2 Bass kernel for nn_LlamaAttention (T=2048, HID=4096, HQ=32, HKV=8, D=128).

Tensor-parallel over heads across 8 NeuronCores: core c owns q-heads 4c..4c+3 and
kv-head c (GQA group size 4 == heads-per-core, so attention is fully core-local).
Wo is row-sharded; each core computes a partial [T, HID] output (transposed) and
the host sums the 8 partials. No device collectives.

All weight/activation inputs are host-pre-swizzled to partition-major layouts so
every DMA is one contiguous run per partition (descriptor-generation on the Sync
engine is the startup bottleneck otherwise).

Self-contained: hardcodes all shapes; builds the Bass kernel once per process.
"""
import numpy as np

T, HID, HQ, HKV, D = 2048, 4096, 32, 8, 128
NCORES = 8
HPC = HQ // NCORES            # 4 q heads per core
QW = HPC * D                  # 512 q columns per core
MW = QW + 2 * D               # 768 qkv columns per core
KO = HID // 128               # 32 k-tiles
NKP = 4                       # phase A K passes
KPQ = KO // NKP               # 8 k-tiles per pass
AC2 = 512                     # phase A T-chunk width
NAC2 = T // AC2               # 4
CH = 512                      # attention tq chunk width
NCH = T // CH                 # 4
SCALING = float(D) ** -0.5

_CACHE = {}


def _build_nc():
    import concourse.mybir as mybir
    import concourse.tile as tile
    from concourse import bacc
    from concourse.bass import _add_dep_helper
    from contextlib import ExitStack

    F32 = mybir.dt.float32
    F32R = mybir.dt.float32r
    AF = mybir.ActivationFunctionType

    nc = bacc.Bacc("TRN2", target_bir_lowering=False, debug=False,
                   dynamic_dma_scratch_size=2048)

    # pre-swizzled inputs (see prep_in_maps)
    hidp = nc.dram_tensor("hidp", [128, NKP * NAC2 * KPQ * AC2], F32R,
                          kind="ExternalInput")
    wqkvp = nc.dram_tensor("wqkvp", [128, KO * MW], F32R, kind="ExternalInput")
    wop = nc.dram_tensor("wop", [128, HPC * (HID // 128) * 128], F32R,
                         kind="ExternalInput")
    cosT = nc.dram_tensor("cosT", [D, T], F32, kind="ExternalInput")
    sinT = nc.dram_tensor("sinT", [D, T], F32, kind="ExternalInput")
    rmatT = nc.dram_tensor("rmatT", [D, D], F32R, kind="ExternalInput")
    onesd = nc.dram_tensor("onesd", [128, 128], F32R, kind="ExternalInput")
    identd = nc.dram_tensor("identd", [128, 128], F32R, kind="ExternalInput")
    masksp = nc.dram_tensor("masksp", [128, 4 * CH], F32, kind="ExternalInput")
    outT = nc.dram_tensor("outT_p", [HID, T], F32, kind="ExternalOutput")

    with tile.TileContext(nc) as tc, ExitStack() as ctx:
        consts = ctx.enter_context(tc.tile_pool(name="consts", bufs=1))
        ones_sb = consts.tile([128, 128], F32R)
        ident = consts.tile([128, 128], F32R)
        rmatT_sb = consts.tile([128, 128], F32R)

        qkv_pool = ctx.enter_context(tc.tile_pool(name="qkvT", bufs=1))
        # m=0..3 q heads, m=4 kT, m=5 vT
        qk_t = [qkv_pool.tile([128, T], F32R, name=f"qkvT{m}") for m in range(6)]
        v_sb = qkv_pool.tile([128, T // 128, 128], F32R, name="v_sb")  # [ts, blk, D]

        csp = ctx.enter_context(tc.tile_pool(name="cs", bufs=1))
        cos_sb = csp.tile([128, T], F32)
        sin_sb = csp.tile([128, T], F32)

        # ---- Phase A: qkvT = wqkv.T @ hidT in NKP K-passes, fp32 partials ----
        with tc.tile_pool(name="w", bufs=2) as wp, \
             tc.tile_pool(name="hid", bufs=3) as hp, \
             tc.tile_pool(name="partial", bufs=1) as pp, \
             tc.tile_pool(name="ropetmp", bufs=2) as tmp_pool, \
             tc.tile_pool(name="psA", bufs=3, space="PSUM") as psA, \
             tc.tile_pool(name="psT", bufs=3, space="PSUM") as psT, \
             tc.tile_pool(name="psR", bufs=2, space="PSUM") as psR:
            part = [pp.tile([128, T], F32, name=f"part{m}") for m in range(6)]

            def load_w(kp, split=False):
                wt = wp.tile([128, KPQ * MW], F32R, tag="w", name=f"w{kp}")
                base = kp * KPQ * MW
                if split:
                    q_ = KPQ * MW // 4
                    for qi in range(4):
                        nc.sync.dma_start(wt[:, qi * q_:(qi + 1) * q_],
                                          wqkvp[:, base + qi * q_:base + (qi + 1) * q_])
                else:
                    nc.sync.dma_start(wt[:], wqkvp[:, base:base + KPQ * MW])
                return wt

            def load_hid(kp, ch, split=False):
                ht = hp.tile([128, KPQ * AC2], F32R, tag="hid", name=f"h{kp}_{ch}")
                base = (kp * NAC2 + ch) * KPQ * AC2
                if split:
                    q_ = KPQ * AC2 // 4
                    for qi in range(4):
                        nc.sync.dma_start(ht[:, qi * q_:(qi + 1) * q_],
                                          hidp[:, base + qi * q_:base + (qi + 1) * q_])
                else:
                    nc.sync.dma_start(ht[:], hidp[:, base:base + KPQ * AC2])
                return ht

            # startup: only the first hid/w quarters go out alone; every
            # other initial DMA waits on them so they get full bandwidth
            # (HWDGE queues fair-share bandwidth across in-flight DMAs).
            first_dmas = []
            ht0 = hp.tile([128, KPQ * AC2], F32R, tag="hid", name="h0_0")
            q_ = KPQ * AC2 // 4
            first_dmas.append(nc.sync.dma_start(ht0[:, :q_], hidp[:, 0:q_]))
            wt0 = wp.tile([128, KPQ * MW], F32R, tag="w", name="w0")
            qw_ = KPQ * MW // 4
            first_dmas.append(nc.sync.dma_start(wt0[:, :qw_], wqkvp[:, 0:qw_]))

            def defer(bi):
                for f in first_dmas:
                    _add_dep_helper(bi.ins, f.ins, sync=True,
                                    reason="defer bulk DMA behind first tiles")
                return bi

            for qi in range(1, 4):
                defer(nc.sync.dma_start(ht0[:, qi * q_:(qi + 1) * q_],
                                        hidp[:, qi * q_:(qi + 1) * q_]))
                defer(nc.sync.dma_start(wt0[:, qi * qw_:(qi + 1) * qw_],
                                        wqkvp[:, qi * qw_:(qi + 1) * qw_]))
            h_cur = ht0
            w_tiles = {0: wt0}
            defer(nc.sync.dma_start(ones_sb[:], onesd[:, :]))
            defer(nc.sync.dma_start(ident[:], identd[:, :]))
            defer(nc.sync.dma_start(rmatT_sb[:], rmatT[:, :]))
            w_tiles[1] = load_w(1)
            for kp in range(NKP):
                w_cur = w_tiles.pop(kp)
                for ch in range(NAC2):
                    asl = slice(ch * AC2, (ch + 1) * AC2)
                    if h_cur is None:
                        h_cur = load_hid(kp, ch)
                    if kp == 2 and ch == 0:
                        nc.sync.dma_start(cos_sb[:], cosT[:, :])
                        nc.sync.dma_start(sin_sb[:], sinT[:, :])
                    if ch == 1 and kp + 2 < NKP:
                        w_tiles[kp + 2] = load_w(kp + 2)
                    for m in range(6):
                        ps = psA.tile([128, AC2], F32, tag="psA")
                        for k in range(KPQ):
                            nc.tensor.matmul(
                                ps[:],
                                w_cur[:, k * MW + m * 128:k * MW + (m + 1) * 128],
                                h_cur[:, k * AC2:(k + 1) * AC2],
                                start=(k == 0), stop=(k == KPQ - 1))
                        if kp == 0:
                            nc.vector.tensor_copy(part[m][:, asl], ps[:])
                        elif kp < NKP - 1:
                            nc.vector.tensor_add(part[m][:, asl], part[m][:, asl],
                                                 ps[:])
                        else:
                            # final pass: single rounding into f32r qkvT
                            nc.vector.tensor_add(qk_t[m][:, asl], part[m][:, asl],
                                                 ps[:])
                            if m < 5:
                                # fused RoPE: x = x*cos + (rmatT.T@x)*sin
                                rp = psR.tile([128, AC2], F32, tag="psR")
                                nc.tensor.matmul(rp[:], rmatT_sb[:], qk_t[m][:, asl],
                                                 start=True, stop=True)
                                tmp = tmp_pool.tile([128, AC2], F32, tag="tmp")
                                nc.vector.tensor_mul(tmp[:], rp[:], sin_sb[:, asl])
                                nc.vector.tensor_mul(qk_t[m][:, asl],
                                                     qk_t[m][:, asl], cos_sb[:, asl])
                                nc.vector.tensor_add(qk_t[m][:, asl],
                                                     qk_t[m][:, asl], tmp[:])
                            else:
                                for jj in range(AC2 // 128):
                                    j = ch * (AC2 // 128) + jj
                                    pst = psT.tile([128, 128], F32R, tag="psT")
                                    nc.tensor.transpose(
                                        pst[:], qk_t[5][:, j * 128:(j + 1) * 128],
                                        ident[:])
                                    nc.vector.tensor_copy(v_sb[:, j, :], pst[:])
                    h_cur = None

        # ---------------- Phase B consts ----------------
        bconsts = ctx.enter_context(tc.tile_pool(name="bconsts", bufs=1))
        mask_sb = bconsts.tile([128, 4 * CH], F32)
        nc.sync.dma_start(mask_sb[:], masksp[:, :])

        wopool = ctx.enter_context(tc.tile_pool(name="wo", bufs=1))
        wo_sb = wopool.tile([128, HPC * (HID // 128) * 128], F32R)
        nc.sync.dma_start(wo_sb[:], wop[:, :])

        probs_pool = ctx.enter_context(tc.tile_pool(name="probs", bufs=6))
        attn_pool = ctx.enter_context(tc.tile_pool(name="attnT", bufs=1))
        attn_sb = [attn_pool.tile([128, T], F32R, name=f"attnT{h}") for h in range(HPC)]
        rden_pool = ctx.enter_context(tc.tile_pool(name="rden", bufs=2))
        out_pool = ctx.enter_context(tc.tile_pool(name="outstage", bufs=6))
        psS = ctx.enter_context(tc.tile_pool(name="psS", bufs=2, space="PSUM"))
        psAcc = ctx.enter_context(tc.tile_pool(name="psAcc", bufs=2, space="PSUM"))
        psD = ctx.enter_context(tc.tile_pool(name="psD", bufs=2, space="PSUM"))
        psO = ctx.enter_context(tc.tile_pool(name="psO", bufs=2, space="PSUM"))

        # ---------------- Phase B+C per tq chunk (C deferred one chunk) ---
        def phase_c(c, mo_range=None):
            sl = slice(c * CH, (c + 1) * CH)
            for mo in (mo_range if mo_range is not None else range(HID // 128)):
                po = psO.tile([128, CH], F32, tag="po")
                for kk in range(HPC):
                    woff = (kk * (HID // 128) + mo) * 128
                    nc.tensor.matmul(po[:], wo_sb[:, woff:woff + 128],
                                     attn_sb[kk][:, sl],
                                     start=(kk == 0), stop=(kk == HPC - 1))
                ob = out_pool.tile([128, CH], F32, tag="ob")
                if mo % 2 == 0:
                    nc.scalar.copy(ob[:], po[:])
                else:
                    nc.vector.tensor_copy(ob[:], po[:])
                nc.sync.dma_start(outT[mo * 128:(mo + 1) * 128, sl], ob[:])

        for c in range(NCH):
            sl = slice(c * CH, (c + 1) * CH)
            nts = (CH // 128) * (c + 1)  # causal: ts tiles 0..nts-1
            for h in range(HPC):
                pa = psAcc.tile([128, CH], F32, tag="acc")
                pd = psD.tile([128, CH], F32, tag="den")
                for j in range(nts):
                    i = j - (CH // 128) * c
                    # diagonal tiles: cols < i*128 fully masked; shrink to
                    # [off:512] (off capped at 256 to keep f32r N>=256 fast)
                    off = 0 if i < 1 else min(i * 128, 256)
                    csl = slice(c * CH + off, (c + 1) * CH)
                    ss = psS.tile([128, CH], F32, tag="psS")
                    nc.tensor.matmul(ss[:, off:], qk_t[4][:, j * 128:(j + 1) * 128],
                                     qk_t[h][:, csl], start=True, stop=True)
                    pr = probs_pool.tile([128, CH], F32R, tag="probs")
                    nc.scalar.activation(pr[:, off:], ss[:, off:], AF.Exp,
                                         scale=SCALING)
                    if i >= 0:
                        nc.vector.tensor_mul(pr[:, off:], pr[:, off:],
                                             mask_sb[:, i * CH + off:(i + 1) * CH])
                    nc.tensor.matmul(pd[:, off:], ones_sb[:], pr[:, off:],
                                     start=(j == 0), stop=(j == nts - 1))
                    nc.tensor.matmul(pa[:, off:], v_sb[:, j, :], pr[:, off:],
                                     start=(j == 0), stop=(j == nts - 1))
                rden = rden_pool.tile([128, CH], F32, tag="rden")
                nc.vector.reciprocal_approx_fast(out=rden[:], in_=pd[:])
                nc.vector.tensor_mul(attn_sb[h][:, sl], pa[:], rden[:])
                if c >= 1:
                    # interleave deferred output-projection work between heads
                    phase_c(c - 1, range(h * 8, (h + 1) * 8))
        phase_c(NCH - 1)

    nc.compile()
    return nc


def get_nc():
    if "nc" not in _CACHE:
        _CACHE["nc"] = _build_nc()
    return _CACHE["nc"]


def prep_in_maps(hidden_states, cos, sin, Wq, Wk, Wv, Wo):
    # hidp[p, (((kp*NAC2)+ch)*KPQ + ko)*AC2 + t] = hidT[(kp*KPQ+ko)*128+p, ch*AC2+t]
    hidT = np.asarray(hidden_states).T  # [HID, T]
    hidp = (np.asarray(hidT).reshape(HID, T)
            .reshape(NKP, KPQ, 128, NAC2, AC2)
            .transpose(2, 0, 3, 1, 4).reshape(128, -1))
    hidp = np.ascontiguousarray(hidp)
    cosT = np.ascontiguousarray(np.asarray(cos).T)
    sinT = np.ascontiguousarray(np.asarray(sin).T)
    # signed rotate-half permutation (as lhsT): rot = rmatT.T @ x
    rmatT = np.zeros((D, D), dtype=np.float32)
    half = D // 2
    rmatT[np.arange(half) + half, np.arange(half)] = -1.0  # rot[d<64] = -x[d+64]
    rmatT[np.arange(half), np.arange(half) + half] = 1.0   # rot[d>=64] = x[d-64]
    # masksp[p, i*CH+f] = 1 if i*128+p <= f
    i_idx = np.arange(4)[None, :, None] * 128
    p_idx = np.arange(128)[:, None, None]
    f_idx = np.arange(CH)[None, None, :]
    masksp = ((i_idx + p_idx) <= f_idx).astype(np.float32).reshape(128, -1)

    in_maps = []
    for c in range(NCORES):
        wqkv = np.concatenate([
            Wq[:, c * QW:(c + 1) * QW],
            Wk[:, c * D:(c + 1) * D],
            Wv[:, c * D:(c + 1) * D],
        ], axis=1)  # [HID, MW]
        # wqkvp[p, ko*MW+m] = wqkv[ko*128+p, m]
        wqkvp = np.ascontiguousarray(
            wqkv.reshape(KO, 128, MW).transpose(1, 0, 2).reshape(128, -1))
        # wop[p, (kk*(HID//128)+mo)*128+q] = Wo[c*QW + kk*128+p, mo*128+q]
        wo_c = Wo[c * QW:(c + 1) * QW, :]
        wop_ = np.ascontiguousarray(
            wo_c.reshape(HPC, 128, HID // 128, 128).transpose(1, 0, 2, 3)
            .reshape(128, -1))
        in_maps.append({
            "hidp": hidp,
            "wqkvp": wqkvp,
            "wop": wop_,
            "cosT": cosT,
            "sinT": sinT,
            "rmatT": rmatT,
            "onesd": np.ones((128, 128), dtype=np.float32),
            "identd": np.eye(128, dtype=np.float32),
            "masksp": masksp,
        })
    return in_maps


def postprocess(results):
    acc = results[0]["outT_p"].copy()
    for r in results[1:]:
        acc += r["outT_p"]
    return np.ascontiguousarray(acc.T).astype(np.float32)


def kernel(hidden_states, position_ids, cos, sin, Wq, Wk, Wv, Wo):
    from concourse.bass_utils import run_bass_kernel_spmd
    hidden_states = np.asarray(hidden_states, dtype=np.float32)
    cos = np.asarray(cos, dtype=np.float32)
    sin = np.asarray(sin, dtype=np.float32)
    Wq = np.asarray(Wq, dtype=np.float32)
    Wk = np.asarray(Wk, dtype=np.float32)
    Wv = np.asarray(Wv, dtype=np.float32)
    Wo = np.asarray(Wo, dtype=np.float32)
    nc = get_nc()
    in_maps = prep_in_maps(hidden_states, cos, sin, Wq, Wk, Wv, Wo)
    res = run_bass_kernel_spmd(nc, in_maps, core_ids=list(range(NCORES)))
    return postprocess(res.results)
